# revision 1
# baseline (speedup 1.0000x reference)
"""Two-layer GCN (PyG GCNConv x2 + ReLU) on 8 Trainium2 NeuronCores.

Strategy: nodes are packed into 128-slot tiles balanced by in-degree and
sharded across cores (49 tiles/core). Each core redundantly computes the
full g1 = dinv * (x @ W1) gather table from a per-core ROTATED copy of x^T
(own nodes at rows 0..6271, so all SPMD addressing is static). Aggregation
is dma_gather of source rows + one-hot (is_equal) matmuls accumulating in
PSUM; self-loops are an identity matmul. Layer-2 input g2 = dinv * (a1 @ W2)
is computed fused per tile and exchanged with a chunked AllGather.
"""

import math
import heapq

import numpy as np
import ml_dtypes

from concourse import bacc, mybir
from concourse.tile import TileContext
from concourse.bass_utils import run_bass_kernel_spmd

BF16 = ml_dtypes.bfloat16
N_CORES = 8

# cost-model predicted makespan (ns) of the last _build_nc, for diagnostics
LAST_PREDICTED_NS = None


def _capture_schedule(tc_cls):
    orig = tc_cls.schedule_and_allocate

    def patched(self, validate_deps=False):
        global LAST_PREDICTED_NS
        r = orig(self, validate_deps)
        try:
            LAST_PREDICTED_NS = int(r[1].time)
        except Exception:
            pass
        return r

    if getattr(tc_cls, "_gnn_patched", False):
        return
    tc_cls.schedule_and_allocate = patched
    tc_cls._gnn_patched = True


_capture_schedule(TileContext)

# Full-problem config. Tests may monkeypatch _CFG before calling kernel().
_CFG = dict(
    N=50000,
    E=800000,
    IN=768,
    HID=512,
    OUT=256,
    T=49,  # tiles per core
)


def _pack_idx(idx_linear):
    """[K*128] int -> [128, K*8] int16 in dma_gather layout."""
    num = idx_linear.shape[0]
    a = idx_linear.reshape(num // 16, 16).T.astype(np.int16)
    return np.tile(a, (8, 1))


def _build_nc(cfg, meta):
    IN, HID, OUT = cfg["IN"], cfg["HID"], cfg["OUT"]
    T = cfg["T"]
    PC = T * 128
    NPAD = PC * N_CORES
    TT = T * N_CORES
    HALF = NPAD // 2
    KL1, KH1, KL2, KH2 = meta["KL1"], meta["KH1"], meta["KL2"], meta["KH2"]
    NK1 = IN // 128
    NK2 = HID // 128
    # AllGather chunks (in tiles per core)
    ch_tiles = meta["ch_tiles"]

    f32 = mybir.dt.float32
    bf = mybir.dt.bfloat16
    i16 = mybir.dt.int16

    nc = bacc.Bacc(None, target_bir_lowering=False, debug=False)
    xT_p = nc.declare_dram_parameter("xT", [IN, NPAD], bf, isOutput=False)
    w1_p = nc.declare_dram_parameter("w1p", [128, NK1 * HID], bf, isOutput=False)
    w2_p = nc.declare_dram_parameter("w2p", [128, NK2 * OUT], bf, isOutput=False)
    b1_p = nc.declare_dram_parameter("b1r", [128, HID], f32, isOutput=False)
    b2_p = nc.declare_dram_parameter("b2r", [128, OUT], f32, isOutput=False)
    iota_p = nc.declare_dram_parameter("iota", [128, 128], f32, isOutput=False)
    ident_p = nc.declare_dram_parameter("identb", [128, 128], bf, isOutput=False)
    dinv_p = nc.declare_dram_parameter("dinvT", [128, TT], f32, isOutput=False)
    idx1_p = nc.declare_dram_parameter("idx1", [T * 128, (KL1 + KH1) * 8], i16, isOutput=False)
    dl1_p = nc.declare_dram_parameter("dl1", [T * 128, KL1 + KH1], f32, isOutput=False)
    idx2_p = nc.declare_dram_parameter("idx2", [T * 128, (KL2 + KH2) * 8], i16, isOutput=False)
    dl2_p = nc.declare_dram_parameter("dl2", [T * 128, KL2 + KH2], f32, isOutput=False)
    out_p = nc.declare_dram_parameter("out", [PC, OUT], f32, isOutput=True)

    with TileContext(nc) as tc:
        with (
            tc.tile_pool(name="const", bufs=1) as cpool,
            tc.tile_pool(name="work", bufs=2) as wpool,
            tc.tile_pool(name="psum", bufs=2, space="PSUM") as ppool,
            tc.tile_pool(name="dram", bufs=1, space="DRAM") as dpool,
        ):
            # ---- internal DRAM ----
            g1d = dpool.tile([NPAD, HID], bf, name="g1d")
            g2s = dpool.tile([PC, OUT], bf, name="g2s")
            cA, cB = ch_tiles
            g2fA = dpool.tile([N_CORES * cA * 128, OUT], bf, name="g2fA", addr_space="Shared")
            g2fB = dpool.tile([N_CORES * cB * 128, OUT], bf, name="g2fB", addr_space="Shared")

            # ---- constants ----
            w1sb = cpool.tile([128, NK1 * HID], bf, name="w1sb")
            nc.sync.dma_start(out=w1sb[:, :], in_=w1_p[:, :])
            w2sb = cpool.tile([128, NK2 * OUT], bf, name="w2sb")
            nc.sync.dma_start(out=w2sb[:, :], in_=w2_p[:, :])
            b1sb = cpool.tile([128, HID], f32, name="b1sb")
            nc.sync.dma_start(out=b1sb[:, :], in_=b1_p[:, :])
            b2sb = cpool.tile([128, OUT], f32, name="b2sb")
            nc.sync.dma_start(out=b2sb[:, :], in_=b2_p[:, :])
            iot = cpool.tile([128, 128], f32, name="iot")
            nc.sync.dma_start(out=iot[:, :], in_=iota_p[:, :])
            idn = cpool.tile([128, 128], bf, name="idn")
            nc.sync.dma_start(out=idn[:, :], in_=ident_p[:, :])
            dnv = cpool.tile([128, TT], f32, name="dnv")
            nc.sync.dma_start(out=dnv[:, :], in_=dinv_p[:, :])

            # ---- phase 1: g1 = dinv * (x @ W1), all NPAD rows ----
            CH = 4  # node tiles per x-chunk
            for ch in range(TT // CH):
                xk = []
                for k in range(NK1):
                    xt = wpool.tile([128, CH * 128], bf, tag=f"xk{k}", bufs=2)
                    nc.sync.dma_start(
                        out=xt[:, :],
                        in_=xT_p[k * 128 : (k + 1) * 128, ch * CH * 128 : (ch + 1) * CH * 128],
                    )
                    xk.append(xt)
                for sub in range(CH):
                    t = ch * CH + sub
                    ps = ppool.tile([128, HID], f32, tag="p1", bufs=2)
                    for k in range(NK1):
                        nc.tensor.matmul(
                            ps[:, :],
                            xk[k][:, sub * 128 : (sub + 1) * 128],
                            w1sb[:, k * HID : (k + 1) * HID],
                            start=(k == 0),
                            stop=(k == NK1 - 1),
                        )
                    ge = wpool.tile([128, HID], bf, tag="ge", bufs=3)
                    nc.vector.tensor_scalar(
                        ge[:, :], ps[:, :], dnv[:, t : t + 1], None, mybir.AluOpType.mult
                    )
                    nc.sync.dma_start(out=g1d[t * 128 : (t + 1) * 128, :], in_=ge[:, :])

            # ---- phase 2: layer-1 aggregation + fused mm2 for own tiles ----
            NB1 = KL1 + KH1
            for t in range(T):
                ix = wpool.tile([128, NB1 * 8], i16, tag="ix1", bufs=2)
                nc.sync.dma_start(out=ix[:, :], in_=idx1_p[t * 128 : (t + 1) * 128, :])
                dl = wpool.tile([128, NB1], f32, tag="dl1", bufs=2)
                nc.sync.dma_start(out=dl[:, :], in_=dl1_p[t * 128 : (t + 1) * 128, :])
                ml = wpool.tile([128, KL1, HID], bf, tag="ml1", bufs=2)
                nc.gpsimd.dma_gather(
                    ml[:, :, :], g1d[0:HALF, :], ix[:, : KL1 * 8],
                    KL1 * 128, KL1 * 128, HID, single_packet=False,
                )
                mh = wpool.tile([128, KH1, HID], bf, tag="mh1", bufs=2)
                nc.gpsimd.dma_gather(
                    mh[:, :, :], g1d[HALF:, :], ix[:, KL1 * 8 :],
                    KH1 * 128, KH1 * 128, HID, single_packet=False,
                )
                gs = wpool.tile([128, HID], bf, tag="gs1", bufs=2)
                nc.sync.dma_start(out=gs[:, :], in_=g1d[t * 128 : (t + 1) * 128, :])

                ps = ppool.tile([128, HID], f32, tag="p1", bufs=2)
                for b in range(NB1):
                    oh = wpool.tile([128, 128], bf, tag="oh", bufs=4)
                    nc.vector.tensor_scalar(
                        oh[:, :], iot[:, :], dl[:, b : b + 1], None,
                        mybir.AluOpType.is_equal,
                    )
                    src = ml[:, b, :] if b < KL1 else mh[:, b - KL1, :]
                    nc.tensor.matmul(ps[:, :], oh[:, :], src, start=(b == 0), stop=False)
                # self-loop: psum += I @ gs
                nc.tensor.matmul(ps[:, :], idn[:, :], gs[:, :], start=False, stop=True)

                t2 = wpool.tile([128, HID], f32, tag="t2", bufs=2)
                nc.vector.tensor_scalar(
                    t2[:, :], ps[:, :], dnv[:, t : t + 1], None, mybir.AluOpType.mult
                )
                t3 = wpool.tile([128, HID], f32, tag="t3", bufs=2)
                nc.vector.tensor_tensor(t3[:, :], t2[:, :], b1sb[:, :], mybir.AluOpType.add)
                a1 = wpool.tile([128, HID], bf, tag="a1", bufs=2)
                nc.scalar.activation(a1[:, :], t3[:, :], mybir.ActivationFunctionType.Relu)

                ps2 = ppool.tile([128, OUT], f32, tag="p2", bufs=2)
                for k in range(NK2):
                    pT = ppool.tile([128, 128], bf, tag="pT", bufs=2)
                    nc.tensor.transpose(pT[:, :], a1[:, k * 128 : (k + 1) * 128], idn[:, :])
                    aT = wpool.tile([128, 128], bf, tag="aT", bufs=2)
                    nc.vector.tensor_copy(aT[:, :], pT[:, :])
                    nc.tensor.matmul(
                        ps2[:, :], aT[:, :], w2sb[:, k * OUT : (k + 1) * OUT],
                        start=(k == 0), stop=(k == NK2 - 1),
                    )
                g2e = wpool.tile([128, OUT], bf, tag="g2e", bufs=3)
                nc.vector.tensor_scalar(
                    g2e[:, :], ps2[:, :], dnv[:, t : t + 1], None, mybir.AluOpType.mult
                )
                nc.sync.dma_start(out=g2s[t * 128 : (t + 1) * 128, :], in_=g2e[:, :])

            # ---- phase 2.5: chunked AllGather of g2 slices (2 chunks = lo/hi) ----
            nc.gpsimd.collective_compute(
                "AllGather",
                mybir.AluOpType.bypass,
                ins=[g2s[0 : cA * 128, :].opt()],
                outs=[g2fA[:, :].opt()],
                replica_groups=[list(range(N_CORES))],
            )
            nc.gpsimd.collective_compute(
                "AllGather",
                mybir.AluOpType.bypass,
                ins=[g2s[cA * 128 :, :].opt()],
                outs=[g2fB[:, :].opt()],
                replica_groups=[list(range(N_CORES))],
            )

            # ---- phase 3: layer-2 aggregation -> output ----
            NB2 = KL2 + KH2
            for t in range(T):
                ix2 = wpool.tile([128, NB2 * 8], i16, tag="ix2", bufs=2)
                nc.sync.dma_start(out=ix2[:, :], in_=idx2_p[t * 128 : (t + 1) * 128, :])
                d2 = wpool.tile([128, NB2], f32, tag="dl2", bufs=2)
                nc.sync.dma_start(out=d2[:, :], in_=dl2_p[t * 128 : (t + 1) * 128, :])
                ml2 = wpool.tile([128, KL2, OUT], bf, tag="ml2", bufs=2)
                nc.gpsimd.dma_gather(
                    ml2[:, :, :], g2fA[:, :], ix2[:, : KL2 * 8],
                    KL2 * 128, KL2 * 128, OUT, single_packet=False,
                )
                mh2 = wpool.tile([128, KH2, OUT], bf, tag="mh2", bufs=2)
                nc.gpsimd.dma_gather(
                    mh2[:, :, :], g2fB[:, :], ix2[:, KL2 * 8 :],
                    KH2 * 128, KH2 * 128, OUT, single_packet=False,
                )
                gs2 = wpool.tile([128, OUT], bf, tag="gs2", bufs=2)
                nc.sync.dma_start(out=gs2[:, :], in_=g2s[t * 128 : (t + 1) * 128, :])

                ps3 = ppool.tile([128, OUT], f32, tag="p2", bufs=2)
                for b in range(NB2):
                    oh2 = wpool.tile([128, 128], bf, tag="oh", bufs=4)
                    nc.vector.tensor_scalar(
                        oh2[:, :], iot[:, :], d2[:, b : b + 1], None,
                        mybir.AluOpType.is_equal,
                    )
                    src = ml2[:, b, :] if b < KL2 else mh2[:, b - KL2, :]
                    nc.tensor.matmul(ps3[:, :], oh2[:, :], src, start=(b == 0), stop=False)
                nc.tensor.matmul(ps3[:, :], idn[:, :], gs2[:, :], start=False, stop=True)

                u2 = wpool.tile([128, OUT], f32, tag="u2", bufs=2)
                nc.vector.tensor_scalar(
                    u2[:, :], ps3[:, :], dnv[:, t : t + 1], None, mybir.AluOpType.mult
                )
                of = wpool.tile([128, OUT], f32, tag="of", bufs=3)
                nc.vector.tensor_tensor(of[:, :], u2[:, :], b2sb[:, :], mybir.AluOpType.add)
                nc.sync.dma_start(out=out_p[t * 128 : (t + 1) * 128, :], in_=of[:, :])

    nc.compile()
    return nc


def _preprocess(x, edge_index, W1, b1, W2, b2, cfg):
    N, E = cfg["N"], cfg["E"]
    IN, HID, OUT = cfg["IN"], cfg["HID"], cfg["OUT"]
    T = cfg["T"]
    PC = T * 128
    NPAD = PC * N_CORES
    TT = T * N_CORES
    HALF = NPAD // 2

    src = np.asarray(edge_index[0], dtype=np.int64)
    dst = np.asarray(edge_index[1], dtype=np.int64)

    indeg = np.bincount(dst, minlength=N)
    deg = indeg.astype(np.float32) + 1.0
    dinv = 1.0 / np.sqrt(deg)

    # ---- balanced node -> (tile, slot) assignment (LPT greedy) ----
    order = np.argsort(-indeg, kind="stable")
    heap = [(0, t, 0) for t in range(TT)]  # (load, tile, used)
    heapq.heapify(heap)
    row_of_node = np.empty(N, dtype=np.int64)
    for n in order:
        load, t, used = heapq.heappop(heap)
        row_of_node[n] = t * 128 + used
        used += 1
        if used < 128 and t * 128 + used < NPAD:
            heapq.heappush(heap, (load + int(indeg[n]), t, used))
    # note: NPAD - N pad slots simply remain unassigned

    node_of_row = np.full(NPAD, -1, dtype=np.int64)
    node_of_row[row_of_node] = np.arange(N)

    # ---- layer-2 chunk-major row mapping ----
    cA = (T + 1) // 2
    ch_tiles = [cA, T - cA]
    ch_off = np.concatenate([[0], np.cumsum(ch_tiles)])  # tile offsets within core
    blk_off = np.concatenate([[0], np.cumsum([N_CORES * c * 128 for c in ch_tiles])])
    SPLIT2 = int(blk_off[1])  # chunk A rows

    rows = np.arange(NPAD)
    r_core = rows // PC
    r_toff = (rows % PC) // 128
    r_slot = rows % 128
    r_chunk = np.searchsorted(ch_off, r_toff, side="right") - 1
    row2_of_row = (
        blk_off[r_chunk]
        + r_core * np.array(ch_tiles)[r_chunk] * 128
        + (r_toff - ch_off[r_chunk]) * 128
        + r_slot
    )

    # ---- per-edge quantities ----
    srow = row_of_node[src]
    drow = row_of_node[dst]
    e_core = drow // PC
    e_toff = (drow % PC) // 128
    e_slot = drow % 128
    srot = (srow - e_core * PC) % NPAD
    lo1 = srot < HALF
    val1 = np.where(lo1, srot, srot - HALF)
    srow2 = row2_of_row[srow]
    lo2 = srow2 < SPLIT2
    val2 = np.where(lo2, srow2, srow2 - SPLIT2)

    # ---- segment counts -> KL/KH ----
    def seg_counts(lo_flag):
        key = (e_core * T + e_toff) * 2 + (~lo_flag).astype(np.int64)
        return np.bincount(key, minlength=TT * 2).reshape(TT, 2)

    cnt1 = seg_counts(lo1)
    cnt2 = seg_counts(lo2)
    KL1 = max(1, math.ceil(cnt1[:, 0].max() / 128))
    KH1 = max(1, math.ceil(cnt1[:, 1].max() / 128))
    KL2 = max(1, math.ceil(cnt2[:, 0].max() / 128))
    KH2 = max(1, math.ceil(cnt2[:, 1].max() / 128))

    # ---- build per-core edge metadata ----
    def build_meta(lo_flag, val, KL, KH, sort_extra):
        NBK = KL + KH
        idx_arr = np.zeros((N_CORES, T, 128, NBK * 8), dtype=np.int16)
        dl_arr = np.full((N_CORES, T, 128, NBK), 999.0, dtype=np.float32)
        ordk = np.lexsort((sort_extra, val, (~lo_flag).astype(np.int64), e_toff, e_core))
        sc, st, sl, sv, ss = (
            e_core[ordk], e_toff[ordk], lo_flag[ordk], val[ordk], e_slot[ordk],
        )
        # segment boundaries
        segkey = (sc * T + st) * 2 + (~sl).astype(np.int64)
        bnd = np.concatenate([[0], np.where(np.diff(segkey) != 0)[0] + 1, [len(segkey)]])
        for i in range(len(bnd) - 1):
            a, b = bnd[i], bnd[i + 1]
            k = segkey[a]
            c, t, h = k // (T * 2), (k // 2) % T, k % 2
            n = b - a
            cap = (KL if h == 0 else KH) * 128
            assert n <= cap
            li = np.zeros(cap, dtype=np.int64)
            li[:n] = sv[a:b]
            dll = np.full(cap, 999.0, dtype=np.float32)
            dll[:n] = ss[a:b]
            colbase = 0 if h == 0 else KL * 8
            nb = cap // 128
            idx_arr[c, t, :, colbase : colbase + nb * 8] = _pack_idx(li)
            bb = 0 if h == 0 else KL
            dl_arr[c, t, :, bb : bb + nb] = dll.reshape(nb, 128).T
        return idx_arr, dl_arr

    idx1, dl1 = build_meta(lo1, val1, KL1, KH1, srow)
    idx2, dl2 = build_meta(lo2, val2, KL2, KH2, srow2)

    # ---- dense host tensors ----
    xPermT = np.zeros((IN, NPAD), dtype=np.float32)
    xPermT[:, row_of_node] = np.asarray(x, dtype=np.float32).T
    dinv_row = np.zeros(NPAD, dtype=np.float32)
    dinv_row[row_of_node] = dinv

    NK1, NK2 = IN // 128, HID // 128
    w1p = (
        np.asarray(W1, np.float32).reshape(NK1, 128, HID).transpose(1, 0, 2).reshape(128, NK1 * HID).astype(BF16)
    )
    w2p = (
        np.asarray(W2, np.float32).reshape(NK2, 128, OUT).transpose(1, 0, 2).reshape(128, NK2 * OUT).astype(BF16)
    )
    b1r = np.tile(np.asarray(b1, np.float32)[None, :], (128, 1))
    b2r = np.tile(np.asarray(b2, np.float32)[None, :], (128, 1))
    iota = np.tile(np.arange(128, dtype=np.float32)[None, :], (128, 1))
    identb = np.eye(128, dtype=np.float32).astype(BF16)

    in_maps = []
    for c in range(N_CORES):
        xr = np.roll(xPermT, -c * PC, axis=1).astype(BF16)
        dr = np.roll(dinv_row, -c * PC)
        dinvT = dr.reshape(TT, 128).T.astype(np.float32).copy()
        in_maps.append(
            {
                "xT": xr,
                "w1p": w1p,
                "w2p": w2p,
                "b1r": b1r,
                "b2r": b2r,
                "iota": iota,
                "identb": identb,
                "dinvT": dinvT,
                "idx1": idx1[c].reshape(cfg["T"] * 128, -1),
                "dl1": dl1[c].reshape(cfg["T"] * 128, -1),
                "idx2": idx2[c].reshape(cfg["T"] * 128, -1),
                "dl2": dl2[c].reshape(cfg["T"] * 128, -1),
            }
        )

    meta = dict(
        KL1=KL1, KH1=KH1, KL2=KL2, KH2=KH2, ch_tiles=ch_tiles, SPLIT2=SPLIT2,
        row_of_node=row_of_node,
    )
    return in_maps, meta


def kernel(x, edge_index, W1, b1, W2, b2):
    cfg = _CFG
    N, OUT = cfg["N"], cfg["OUT"]
    PC = cfg["T"] * 128
    in_maps, meta = _preprocess(x, edge_index, W1, b1, W2, b2, cfg)
    nc = _build_nc(cfg, meta)
    import os
    if os.environ.get("GNN_SIM"):
        from concourse import bass_interp

        sim = bass_interp.MultiCoreSim(nc, N_CORES)
        for c in range(N_CORES):
            for k, v in in_maps[c].items():
                sim.cores[c].tensor(k)[:] = v
        sim.simulate()
        results = [
            {"out": np.array(sim.cores[c].tensor("out"))} for c in range(N_CORES)
        ]
    else:
        res = run_bass_kernel_spmd(nc, in_maps, core_ids=list(range(N_CORES)))
        results = res.results
    out = np.empty((N, OUT), dtype=np.float32)
    row = meta["row_of_node"]
    core = row // PC
    local = row % PC
    for c in range(N_CORES):
        m = core == c
        out[np.where(m)[0]] = results[c]["out"][local[m]]
    return out



# revision 14
# speedup vs baseline: 1.1105x; 1.1105x over previous
"""Two-layer GCN (PyG GCNConv x2 + ReLU) on 8 Trainium2 NeuronCores.

v2: nodes packed into 128-slot tiles balanced by in-degree, sharded across
cores (49 tiles/core), per-core ROTATED row order so SPMD addressing is
static. Layer 1: every core redundantly computes the full message table
g1 = 16*dinv_s*(x@W1) in fp8 with DoubleRow matmuls (x pre-scaled by dinv
on host, W1 pre-scaled x64), then aggregates its own tiles with fp8
dma_gathers + merged one-hot (single broadcast-AP is_equal per tile) +
DoubleRow pair matmuls in TRANSPOSED orientation (psum holds h^T), so mm2
needs no transposes: a1T = relu(hT) feeds W2 directly. All GCN norms are
algebraically folded: g2 = dinv_d^2/16 * (relu(hT)^T @ W2) equals
dinv*(a1@W2) exactly. Layer 2 stays bf16: single AllGather of g2, then
gather + one-hot matmul aggregation as in v1.
"""

import math
import heapq

import numpy as np
import ml_dtypes

from concourse import bacc, mybir
from concourse.bass import AP
from concourse.tile import TileContext
from concourse.bass_utils import run_bass_kernel_spmd

BF16 = ml_dtypes.bfloat16
FP8 = ml_dtypes.float8_e4m3fn
N_CORES = 8

# cost-model predicted makespan (ns) of the last _build_nc, for diagnostics
LAST_PREDICTED_NS = None


def _capture_schedule(tc_cls):
    orig = tc_cls.schedule_and_allocate

    def patched(self, validate_deps=False):
        global LAST_PREDICTED_NS
        r = orig(self, validate_deps)
        try:
            LAST_PREDICTED_NS = int(r[1].time)
        except Exception:
            pass
        return r

    if getattr(tc_cls, "_gnn_patched", False):
        return
    tc_cls.schedule_and_allocate = patched
    tc_cls._gnn_patched = True


_capture_schedule(TileContext)

_CFG = dict(
    N=50000,
    E=800000,
    IN=768,
    HID=512,
    OUT=256,
    T=49,  # tiles per core
)


def _pack_idx(idx_linear):
    """[K*128] int -> [128, K*8] int16 in dma_gather layout."""
    num = idx_linear.shape[0]
    a = idx_linear.reshape(num // 16, 16).T.astype(np.int16)
    return np.tile(a, (8, 1))


def _build_nc(cfg, meta):
    IN, HID, OUT = cfg["IN"], cfg["HID"], cfg["OUT"]
    T = cfg["T"]
    PC = T * 128
    NPAD = PC * N_CORES
    TT = T * N_CORES
    HALF = NPAD // 2
    KL1, KH1, KL2, KH2 = meta["KL1"], meta["KH1"], meta["KL2"], meta["KH2"]
    NB1 = KL1 + KH1  # even
    NB2 = KL2 + KH2
    NG1 = IN // 256  # DoubleRow k-groups per term in phase 1
    # residual-pair phase 1: h = x1@w1 + x2@w1 + x1@w2 (fp8 hi/lo splits)
    XIDX = list(range(NG1)) + list(range(NG1, 2 * NG1)) + list(range(NG1))
    WIDX = list(range(NG1)) + list(range(NG1)) + list(range(NG1, 2 * NG1))
    NMM1 = len(XIDX)  # 9
    NC2 = HID // 128  # feature chunks of hT / a1T
    has_b1 = meta["has_b1"]
    has_b2 = meta["has_b2"]

    f32 = mybir.dt.float32
    bf = mybir.dt.bfloat16
    f8 = mybir.dt.float8e4
    i16 = mybir.dt.int16

    CH = 4  # node tiles per phase-1 x chunk / g1d write batch
    GB = 8  # tiles per idx/dl load batch

    nc = bacc.Bacc(None, target_bir_lowering=False, debug=False)
    xdr_p = nc.declare_dram_parameter("xdr", [128, TT * 2 * IN], f8, isOutput=False)
    w1_p = nc.declare_dram_parameter("w1p", [128, 2 * NG1 * 2 * HID], f8, isOutput=False)
    w2_p = nc.declare_dram_parameter("w2p", [128, NC2 * OUT], bf, isOutput=False)
    iota_p = nc.declare_dram_parameter("iota", [128, 128], bf, isOutput=False)
    id8_p = nc.declare_dram_parameter("id8", [128, 128], f8, isOutput=False)
    idb_p = nc.declare_dram_parameter("idb", [128, 128], bf, isOutput=False)
    dnv2_p = nc.declare_dram_parameter("dnv2", [128, T], f32, isOutput=False)
    dnv3_p = nc.declare_dram_parameter("dnv3", [128, T], f32, isOutput=False)
    idx1_p = nc.declare_dram_parameter("idx1", [128, T * NB1 * 8], i16, isOutput=False)
    dl1_p = nc.declare_dram_parameter("dl1", [128, T * NB1], bf, isOutput=False)
    idx2_p = nc.declare_dram_parameter("idx2", [128, T * NB2 * 8], i16, isOutput=False)
    dl2_p = nc.declare_dram_parameter("dl2", [128, T * NB2], bf, isOutput=False)
    if has_b1:
        b1_p = nc.declare_dram_parameter("b1p", [1, HID], bf, isOutput=False)
        binv_p = nc.declare_dram_parameter("binv", [1, T * 128], bf, isOutput=False)
    if has_b2:
        b2_p = nc.declare_dram_parameter("b2r", [128, OUT], f32, isOutput=False)
    out_p = nc.declare_dram_parameter("out", [PC, OUT], f32, isOutput=True)

    with TileContext(nc) as tc:
        with (
            tc.tile_pool(name="const", bufs=1) as cpool,
            tc.tile_pool(name="work", bufs=2) as wpool,
            tc.tile_pool(name="psum", bufs=2, space="PSUM") as ppool,
            tc.tile_pool(name="dram", bufs=1, space="DRAM") as dpool,
        ):
            # ---- internal DRAM ----
            g1d = dpool.tile([NPAD, HID], f8, name="g1d")
            g2s = dpool.tile([PC, OUT], bf, name="g2s")
            g2f = dpool.tile([NPAD, OUT], bf, name="g2f", addr_space="Shared")

            # ---- constants ----
            w1sb = cpool.tile([128, 2 * NG1, 2, HID], f8, name="w1sb")
            nc.sync.dma_start(out=w1sb[:, :, :, :], in_=w1_p[:, :])
            w2sb = cpool.tile([128, NC2, OUT], bf, name="w2sb")
            nc.sync.dma_start(out=w2sb[:, :, :], in_=w2_p[:, :])
            iot = cpool.tile([128, 128], bf, name="iot")
            nc.sync.dma_start(out=iot[:, :], in_=iota_p[:, :])
            id8 = cpool.tile([128, 128], f8, name="id8")
            nc.sync.dma_start(out=id8[:, :], in_=id8_p[:, :])
            idb = cpool.tile([128, 128], bf, name="idb")
            nc.sync.dma_start(out=idb[:, :], in_=idb_p[:, :])
            dn2 = cpool.tile([128, T], f32, name="dn2")
            nc.sync.dma_start(out=dn2[:, :], in_=dnv2_p[:, :])
            dn3 = cpool.tile([128, T], f32, name="dn3")
            nc.sync.dma_start(out=dn3[:, :], in_=dnv3_p[:, :])
            if has_b1:
                b1sb = cpool.tile([1, HID], bf, name="b1sb")
                nc.sync.dma_start(out=b1sb[:, :], in_=b1_p[:, :])
                bnv = cpool.tile([1, T * 128], bf, name="bnv")
                nc.sync.dma_start(out=bnv[:, :], in_=binv_p[:, :])
            if has_b2:
                b2sb = cpool.tile([128, OUT], f32, name="b2sb")
                nc.sync.dma_start(out=b2sb[:, :], in_=b2_p[:, :])

            # ---- phase 1: g1 = 16*dinv_s*(x@W1) in fp8, all NPAD rows ----
            for ch in range(TT // CH):
                xt = wpool.tile([128, CH, 2 * NG1, 2, 128], f8, tag="xdr", bufs=2)
                nc.sync.dma_start(
                    out=xt[:, :, :, :, :],
                    in_=xdr_p[:, ch * CH * 2 * IN : (ch + 1) * CH * 2 * IN],
                )
                ge4 = wpool.tile([128, CH, HID], f8, tag="ge4", bufs=2)
                for sub in range(CH):
                    t = ch * CH + sub
                    ps = ppool.tile([128, HID], f32, tag="p1", bufs=2)
                    for j in range(NMM1):
                        nc.tensor.matmul(
                            ps[:, :],
                            xt[:, sub, XIDX[j], :, :],
                            w1sb[:, WIDX[j], :, :],
                            start=(j == 0),
                            stop=(j == NMM1 - 1),
                            perf_mode=mybir.MatmulPerfMode.DoubleRow,
                        )
                    # store 0.25*psum = 16*dinv_s*(x@W1); alternate DVE/Act
                    if sub % 2 == 0:
                        nc.vector.tensor_scalar(
                            ge4[:, sub, :], ps[:, :], 0.25, None, mybir.AluOpType.mult
                        )
                    else:
                        nc.scalar.activation(
                            ge4[:, sub, :], ps[:, :],
                            mybir.ActivationFunctionType.Copy, 0.0, 0.25,
                        )
                # DRAM AP iterating (partition, sub-tile, col) to match ge4
                o_ap = g1d[ch * CH * 128 : (ch + 1) * CH * 128, :]
                o2 = AP(
                    o_ap.tensor, o_ap.offset,
                    [[HID, 128], [128 * HID, CH], [1, HID]],
                )
                nc.sync.dma_start(out=o2, in_=ge4[:, :, :])

            # ---- phase 2: L1 aggregation (transposed) + fused mm2 ----
            nib = math.ceil(T / GB)
            ix1b = [None] * nib
            dl1b = [None] * nib
            ix2b = [None] * nib
            dl2b = [None] * nib
            for i in range(nib):
                lo, hi = i * GB, min((i + 1) * GB, T)
                n = hi - lo
                ix1b[i] = wpool.tile([128, GB, NB1 * 8], i16, tag="ix1", bufs=2, name=f"ix1b{i}")
                nc.sync.dma_start(
                    out=ix1b[i][:, :n, :], in_=idx1_p[:, lo * NB1 * 8 : hi * NB1 * 8]
                )
                dl1b[i] = wpool.tile([128, GB, NB1], bf, tag="dl1", bufs=2, name=f"dl1b{i}")
                nc.sync.dma_start(
                    out=dl1b[i][:, :n, :], in_=dl1_p[:, lo * NB1 : hi * NB1]
                )

            for t in range(T):
                bi, bo = t // GB, t % GB
                m1 = wpool.tile([128, NB1, HID], f8, tag="m1", bufs=2)
                nc.gpsimd.dma_gather(
                    m1[:, :KL1, :], g1d[0:HALF, :], ix1b[bi][:, bo, : KL1 * 8],
                    KL1 * 128, KL1 * 128, HID, single_packet=False,
                )
                nc.gpsimd.dma_gather(
                    m1[:, KL1:, :], g1d[HALF:, :], ix1b[bi][:, bo, KL1 * 8 :],
                    KH1 * 128, KH1 * 128, HID, single_packet=False,
                )
                gs = wpool.tile([128, HID], f8, tag="gs1", bufs=2)
                nc.sync.dma_start(out=gs[:, :], in_=g1d[t * 128 : (t + 1) * 128, :])

                oh = wpool.tile([128, NB1, 128], f8, tag="oh1", bufs=2)
                i_ap = iot[:, :]
                i_bc = AP(i_ap.tensor, i_ap.offset, [i_ap.ap[0], [0, NB1], i_ap.ap[1]])
                d_ap = dl1b[bi][:, bo, :]
                d_bc = AP(d_ap.tensor, d_ap.offset, list(d_ap.ap) + [[0, 128]])
                nc.vector.tensor_tensor(oh[:, :, :], i_bc, d_bc, mybir.AluOpType.is_equal)

                hT = ppool.tile([128, NC2, 128], f32, tag="hT", bufs=2)
                for k in range(NC2):
                    # self-loop: hT_k += gs_k^T  (gs rows are the tile's nodes)
                    nc.tensor.matmul(
                        hT[:, k, :], gs[:, k * 128 : (k + 1) * 128], id8[:, :],
                        start=True, stop=False,
                    )
                    if has_b1:
                        nc.tensor.matmul(
                            hT[:, k, :], b1sb[:, k * 128 : (k + 1) * 128],
                            bnv[:, t * 128 : (t + 1) * 128],
                            start=False, stop=False,
                        )
                    for p in range(NB1 // 2):
                        nc.tensor.matmul(
                            hT[:, k, :],
                            m1[:, 2 * p : 2 * p + 2, k * 128 : (k + 1) * 128],
                            oh[:, 2 * p : 2 * p + 2, :],
                            start=False, stop=(p == NB1 // 2 - 1),
                            perf_mode=mybir.MatmulPerfMode.DoubleRow,
                        )
                a1T = wpool.tile([128, NC2, 128], bf, tag="a1T", bufs=2)
                for k in range(NC2):
                    nc.scalar.activation(
                        a1T[:, k, :], hT[:, k, :], mybir.ActivationFunctionType.Relu
                    )
                ps2 = ppool.tile([128, OUT], f32, tag="p2", bufs=2)
                for k in range(NC2):
                    nc.tensor.matmul(
                        ps2[:, :], a1T[:, k, :], w2sb[:, k, :],
                        start=(k == 0), stop=(k == NC2 - 1),
                    )
                g2e = wpool.tile([128, OUT], bf, tag="g2e", bufs=3)
                nc.vector.tensor_scalar(
                    g2e[:, :], ps2[:, :], dn2[:, t : t + 1], None, mybir.AluOpType.mult
                )
                nc.sync.dma_start(out=g2s[t * 128 : (t + 1) * 128, :], in_=g2e[:, :])

            # ---- phase 2.5: single AllGather of g2 ----
            nc.gpsimd.collective_compute(
                "AllGather",
                mybir.AluOpType.bypass,
                ins=[g2s[:, :].opt()],
                outs=[g2f[:, :].opt()],
                replica_groups=[list(range(N_CORES))],
            )

            # ---- phase 3: L2 aggregation -> output ----
            for i in range(nib):
                lo, hi = i * GB, min((i + 1) * GB, T)
                n = hi - lo
                ix2b[i] = wpool.tile([128, GB, NB2 * 8], i16, tag="ix2", bufs=2, name=f"ix2b{i}")
                nc.sync.dma_start(
                    out=ix2b[i][:, :n, :], in_=idx2_p[:, lo * NB2 * 8 : hi * NB2 * 8]
                )
                dl2b[i] = wpool.tile([128, GB, NB2], bf, tag="dl2", bufs=2, name=f"dl2b{i}")
                nc.sync.dma_start(
                    out=dl2b[i][:, :n, :], in_=dl2_p[:, lo * NB2 : hi * NB2]
                )

            for t in range(T):
                bi, bo = t // GB, t % GB
                m2 = wpool.tile([128, NB2, OUT], bf, tag="m2", bufs=2)
                nc.gpsimd.dma_gather(
                    m2[:, :KL2, :], g2f[0:HALF, :], ix2b[bi][:, bo, : KL2 * 8],
                    KL2 * 128, KL2 * 128, OUT, single_packet=False,
                )
                nc.gpsimd.dma_gather(
                    m2[:, KL2:, :], g2f[HALF:, :], ix2b[bi][:, bo, KL2 * 8 :],
                    KH2 * 128, KH2 * 128, OUT, single_packet=False,
                )
                gs2 = wpool.tile([128, OUT], bf, tag="gs2", bufs=2)
                nc.sync.dma_start(out=gs2[:, :], in_=g2s[t * 128 : (t + 1) * 128, :])

                oh2 = wpool.tile([128, NB2, 128], bf, tag="oh2", bufs=2)
                i_ap = iot[:, :]
                i_bc = AP(i_ap.tensor, i_ap.offset, [i_ap.ap[0], [0, NB2], i_ap.ap[1]])
                d_ap = dl2b[bi][:, bo, :]
                d_bc = AP(d_ap.tensor, d_ap.offset, list(d_ap.ap) + [[0, 128]])
                nc.vector.tensor_tensor(oh2[:, :, :], i_bc, d_bc, mybir.AluOpType.is_equal)

                ps3 = ppool.tile([128, OUT], f32, tag="p3", bufs=2)
                nc.tensor.matmul(ps3[:, :], idb[:, :], gs2[:, :], start=True, stop=False)
                for b in range(NB2):
                    nc.tensor.matmul(
                        ps3[:, :], oh2[:, b, :], m2[:, b, :],
                        start=False, stop=(b == NB2 - 1),
                    )
                of = wpool.tile([128, OUT], f32, tag="of", bufs=3)
                nc.vector.tensor_scalar(
                    of[:, :], ps3[:, :], dn3[:, t : t + 1], None, mybir.AluOpType.mult
                )
                if has_b2:
                    nc.vector.tensor_tensor(
                        of[:, :], of[:, :], b2sb[:, :], mybir.AluOpType.add
                    )
                nc.sync.dma_start(out=out_p[t * 128 : (t + 1) * 128, :], in_=of[:, :])

    nc.compile()
    return nc


def _preprocess(x, edge_index, W1, b1, W2, b2, cfg):
    N, E = cfg["N"], cfg["E"]
    IN, HID, OUT = cfg["IN"], cfg["HID"], cfg["OUT"]
    T = cfg["T"]
    PC = T * 128
    NPAD = PC * N_CORES
    TT = T * N_CORES
    HALF = NPAD // 2
    NG1 = IN // 256
    NC2 = HID // 128

    src = np.asarray(edge_index[0], dtype=np.int64)
    dst = np.asarray(edge_index[1], dtype=np.int64)

    indeg = np.bincount(dst, minlength=N)
    deg = indeg.astype(np.float32) + 1.0
    dinv = 1.0 / np.sqrt(deg)

    # ---- balanced node -> (tile, slot) assignment (LPT greedy) ----
    order = np.argsort(-indeg, kind="stable")
    heap = [(0, t, 0) for t in range(TT)]
    heapq.heapify(heap)
    row_of_node = np.empty(N, dtype=np.int64)
    for n in order:
        load, t, used = heapq.heappop(heap)
        row_of_node[n] = t * 128 + used
        used += 1
        if used < 128 and t * 128 + used < NPAD:
            heapq.heappush(heap, (load + int(indeg[n]), t, used))

    # ---- per-edge quantities ----
    srow = row_of_node[src]
    drow = row_of_node[dst]
    e_core = drow // PC
    e_toff = (drow % PC) // 128
    e_slot = drow % 128
    srot = (srow - e_core * PC) % NPAD  # layer-1 source rows, rotated per core
    lo1 = srot < HALF
    val1 = np.where(lo1, srot, srot - HALF)
    lo2 = srow < HALF  # layer-2 source rows, natural order
    val2 = np.where(lo2, srow, srow - HALF)

    def seg_counts(lo_flag):
        key = (e_core * T + e_toff) * 2 + (~lo_flag).astype(np.int64)
        return np.bincount(key, minlength=TT * 2).reshape(TT, 2)

    cnt1 = seg_counts(lo1)
    cnt2 = seg_counts(lo2)
    KL1 = max(1, math.ceil(cnt1[:, 0].max() / 128))
    KH1 = max(1, math.ceil(cnt1[:, 1].max() / 128))
    if (KL1 + KH1) % 2:
        KH1 += 1
    KL2 = max(1, math.ceil(cnt2[:, 0].max() / 128))
    KH2 = max(1, math.ceil(cnt2[:, 1].max() / 128))

    def build_meta_tbl(lo_flag, val, KL, KH, sort_extra):
        NBK = KL + KH
        idx_arr = np.zeros((N_CORES, T, 128, NBK * 8), dtype=np.int16)
        dl_arr = np.full((N_CORES, T, 128, NBK), 999.0, dtype=np.float32)
        ordk = np.lexsort((sort_extra, val, (~lo_flag).astype(np.int64), e_toff, e_core))
        sc, st, sl, sv, ss = (
            e_core[ordk], e_toff[ordk], lo_flag[ordk], val[ordk], e_slot[ordk],
        )
        segkey = (sc * T + st) * 2 + (~sl).astype(np.int64)
        bnd = np.concatenate([[0], np.where(np.diff(segkey) != 0)[0] + 1, [len(segkey)]])
        for i in range(len(bnd) - 1):
            a, b = bnd[i], bnd[i + 1]
            k = segkey[a]
            c, t, h = k // (T * 2), (k // 2) % T, k % 2
            n = b - a
            cap = (KL if h == 0 else KH) * 128
            assert n <= cap
            li = np.zeros(cap, dtype=np.int64)
            li[:n] = sv[a:b]
            dll = np.full(cap, 999.0, dtype=np.float32)
            dll[:n] = ss[a:b]
            colbase = 0 if h == 0 else KL * 8
            nb = cap // 128
            idx_arr[c, t, :, colbase : colbase + nb * 8] = _pack_idx(li)
            bb = 0 if h == 0 else KL
            dl_arr[c, t, :, bb : bb + nb] = dll.reshape(nb, 128).T
        return idx_arr, dl_arr

    idx1, dl1 = build_meta_tbl(lo1, val1, KL1, KH1, srow)
    idx2, dl2 = build_meta_tbl(lo2, val2, KL2, KH2, srow)

    # ---- dense host tensors ----
    xs = np.asarray(x, dtype=np.float32) * dinv[:, None]  # dinv_s pre-fold
    xPermT = np.zeros((IN, NPAD), dtype=np.float32)
    xPermT[:, row_of_node] = xs.T
    dinv_row = np.zeros(NPAD, dtype=np.float32)
    dinv_row[row_of_node] = dinv

    wsc = np.asarray(W1, np.float32) * 64.0
    w1hi = wsc.astype(FP8)
    w1lo = (wsc - w1hi.astype(np.float32)).astype(FP8)
    w1cat = np.concatenate(
        [w1hi.astype(np.float32), w1lo.astype(np.float32)], axis=0
    ).reshape(2 * NG1, 2, 128, HID)
    w1p = w1cat.transpose(2, 0, 1, 3).reshape(128, 2 * NG1 * 2 * HID).astype(FP8)
    w2p = (
        np.asarray(W2, np.float32).reshape(NC2, 128, OUT).transpose(1, 0, 2)
        .reshape(128, NC2 * OUT).astype(BF16)
    )
    iota = np.tile(np.arange(128, dtype=np.float32)[None, :], (128, 1)).astype(BF16)
    id8 = np.eye(128, dtype=np.float32).astype(FP8)
    idb = np.eye(128, dtype=np.float32).astype(BF16)

    b1np = np.asarray(b1, np.float32)
    b2np = np.asarray(b2, np.float32)
    has_b1 = bool(np.any(b1np != 0))
    has_b2 = bool(np.any(b2np != 0))

    xhi = xPermT.astype(FP8)
    xlo = (xPermT - xhi.astype(np.float32)).astype(FP8)

    in_maps = []
    for c in range(N_CORES):
        # [2*IN, NPAD] rotated: rows 0:IN = x_hi, IN:2*IN = x_lo
        xr = np.concatenate(
            [
                np.roll(xhi.astype(np.float32), -c * PC, axis=1),
                np.roll(xlo.astype(np.float32), -c * PC, axis=1),
            ],
            axis=0,
        )
        # DoubleRow pack: xdr[k, tt, g, i, m] = xr[g*256+i*128+k, tt*128+m]
        xdr = (
            xr.reshape(2 * NG1, 2, 128, TT, 128).transpose(2, 3, 0, 1, 4)
            .reshape(128, TT * 2 * IN).astype(FP8)
        )
        dr = np.roll(dinv_row, -c * PC)[:PC]  # own rows
        dslots = dr.reshape(T, 128).T.astype(np.float32)
        dnv2 = (dslots * dslots) / 16.0
        dnv3 = dslots.copy()
        m = {
            "xdr": xdr,
            "w1p": w1p,
            "w2p": w2p,
            "iota": iota,
            "id8": id8,
            "idb": idb,
            "dnv2": dnv2,
            "dnv3": dnv3,
            "idx1": idx1[c].transpose(1, 0, 2).reshape(128, -1),
            "dl1": dl1[c].transpose(1, 0, 2).reshape(128, -1).astype(BF16),
            "idx2": idx2[c].transpose(1, 0, 2).reshape(128, -1),
            "dl2": dl2[c].transpose(1, 0, 2).reshape(128, -1).astype(BF16),
        }
        if has_b1:
            m["b1p"] = b1np[None, :].astype(BF16)
            binv = np.where(dr > 0, 16.0 / np.maximum(dr, 1e-30), 0.0)
            m["binv"] = binv[None, :].astype(BF16)
        if has_b2:
            m["b2r"] = np.tile(b2np[None, :], (128, 1))
        in_maps.append(m)

    meta = dict(
        KL1=KL1, KH1=KH1, KL2=KL2, KH2=KH2,
        has_b1=has_b1, has_b2=has_b2,
        row_of_node=row_of_node,
    )
    return in_maps, meta


def kernel(x, edge_index, W1, b1, W2, b2):
    cfg = _CFG
    N, OUT = cfg["N"], cfg["OUT"]
    PC = cfg["T"] * 128
    in_maps, meta = _preprocess(x, edge_index, W1, b1, W2, b2, cfg)
    nc = _build_nc(cfg, meta)
    import os
    if os.environ.get("GNN_SIM"):
        from concourse import bass_interp

        sim = bass_interp.MultiCoreSim(nc, N_CORES)
        for c in range(N_CORES):
            for k, v in in_maps[c].items():
                sim.cores[c].tensor(k)[:] = v
        sim.simulate()
        results = [
            {"out": np.array(sim.cores[c].tensor("out"))} for c in range(N_CORES)
        ]
    else:
        res = run_bass_kernel_spmd(nc, in_maps, core_ids=list(range(N_CORES)))
        results = res.results
    out = np.empty((N, OUT), dtype=np.float32)
    row = meta["row_of_node"]
    core = row // PC
    local = row % PC
    for c in range(N_CORES):
        m = core == c
        out[np.where(m)[0]] = results[c]["out"][local[m]]
    return out
